# revision 49
# baseline (speedup 1.0000x reference)
"""Trainium2 Bass kernel for 8-head causal self-attention (b=2, s=4096, d=512, 8 heads x 64).

Sharding: 8 cores = 2 (batch) x 4 (head-pair). Core c handles batch c//4 and heads
(2*(c%4), 2*(c%4)+1). Each core computes a partial output projection over its two
heads' columns of W_O; the host sums the 4 partials per batch (tensor-parallel
all-reduce done on host at gather time).

Final structure (v15): one flat software-pipelined stream over all 144
(q-tile, p-block) blocks; K^T stored zero-padded per head (kTA/kTB) so score
matmuls run at full 128-contraction rate; PV runs 3 blocks behind its block's
exp (PV_LAG) so it never touches the pt tile ACT is writing; the PV-accumulator
PSUM tile is drained right after the last PV matmul (sums row first, then two
partition-shifted DVE casts) so the next q-tile's PV never waits on the single
zps buffer; the rest of the softmax normalization (ones-row PE broadcast of
sums, fast reciprocal, one full-width scale) is deferred into block 7 of the
next q-tile; K(qt+1) is projected as one boundary burst; Q/V builds for qt+1
and the previous tile's output projection are spread one-per-block through a
persistent filler queue; DMA issue order puts W_K and xT g=0 first (each
dma_start costs ~650ns serially on the Sync engine).

Per-core algorithm ("everything transposed", softmax over the partition axis):
  - K^T/Q^T projections from xT (2 heads packed), V re-transposed to [p, 65]
    tiles with a fused ones-column (softmax sums ride the PV matmul)
  - S^T[p,q] blocks via row-tiled matmuls, exp on ScalarE (scale=1/8),
    causal 128x128 diagonal masked post-exp with one triangular bf16 mask
  - PV matmul accumulates z^T (+ sums row via the ones-column) in PSUM
"""

import numpy as np
import ml_dtypes
from collections import deque
from contextlib import ExitStack

import concourse.bass as bass
import concourse.mybir as mybir
import concourse.tile as tile
from concourse import bacc
from concourse.bass import ts, ds
from concourse.masks import make_identity

BF16 = mybir.dt.bfloat16
F32 = mybir.dt.float32

B, S, D, NH, DH = 2, 4096, 512, 8, 64
N_CORES = 8
QT = 512          # q tile (free dim of S^T blocks)
PC = 128          # p chunk (partition dim of S^T blocks)


def build_attention_core(s=S, d=D, dh=DH):
    nqt = s // QT
    n_kc = d // 128
    nc = bacc.Bacc()
    xT_dram = nc.dram_tensor("xT", [d, s], BF16, kind="ExternalInput")
    wkT_dram = nc.dram_tensor("wkT", [d, 2 * dh], BF16, kind="ExternalInput")
    wqT_dram = nc.dram_tensor("wqT", [d, 2 * dh], BF16, kind="ExternalInput")
    wvT_dram = nc.dram_tensor("wvT", [d, 2 * dh], BF16, kind="ExternalInput")
    woT_dram = nc.dram_tensor("woT", [2 * dh, d], BF16, kind="ExternalInput")
    out_dram = nc.dram_tensor("out", [s, d], BF16, kind="ExternalOutput")

    with ExitStack() as ctx:
        tc = ctx.enter_context(tile.TileContext(nc))
        consts = ctx.enter_context(tc.tile_pool(name="consts", bufs=1))
        acts = ctx.enter_context(tc.tile_pool(name="acts", bufs=1))
        vstage = ctx.enter_context(tc.tile_pool(name="vstage", bufs=2))
        ptp = ctx.enter_context(tc.tile_pool(name="ptp", bufs=5))
        nrm = ctx.enter_context(tc.tile_pool(name="nrm", bufs=3))
        ost = ctx.enter_context(tc.tile_pool(name="ost", bufs=4))
        psp = ctx.enter_context(tc.tile_pool(name="psp", bufs=2, space="PSUM"))
        pzp = ctx.enter_context(tc.tile_pool(name="pzp", bufs=1, space="PSUM"))
        pmp = ctx.enter_context(tc.tile_pool(name="pmp", bufs=2, space="PSUM"))

        # ---- constants ----
        ident_bf = consts.tile([128, 128], BF16, tag="idb")
        make_identity(nc, ident_bf[:])
        ones128 = consts.tile([128, dh], BF16, tag="ones")
        nc.gpsimd.memset(ones128[:], 1.0)
        # diagonal 128x128 causal mask: keep[p, q] = 1 if q >= p
        dmask = consts.tile([128, 128], BF16, tag="dgm")
        nc.gpsimd.memset(dmask[:], 1.0)
        nc.gpsimd.affine_select(
            out=dmask[:], in_=dmask[:],
            compare_op=mybir.AluOpType.is_ge,
            fill=0.0, base=0,
            pattern=[[1, 128]], channel_multiplier=-1,
        )

        # ---- persistent activations / weights ----
        xT = acts.tile([128, n_kc, s], BF16, tag="xT")
        # K^T stored zero-padded per head so score matmuls run at full
        # 128-contraction rate: kTA rows 0-63 = K_A (rows 64-127 zero),
        # kTB rows 64-127 = K_B (rows 0-63 zero). The zero rows multiply
        # the other head's rows of qT and contribute nothing.
        kTA = acts.tile([128, s], BF16, tag="kTA")
        kTB = acts.tile([128, s], BF16, tag="kTB")
        nc.vector.memset(kTA[ds(dh, dh), :], 0.0)
        nc.gpsimd.memset(kTB[0:dh, :], 0.0)
        qT = acts.tile([128, s], BF16, tag="qT")
        vtiles = acts.tile([128, s // PC, 2 * (dh + 1)], BF16, tag="vt")
        wk_sb = acts.tile([128, n_kc, 2 * dh], BF16, tag="wk")
        wq_sb = acts.tile([128, n_kc, 2 * dh], BF16, tag="wq")
        wv_sb = acts.tile([128, n_kc, 2 * dh], BF16, tag="wv")
        wo_sb = acts.tile([128, d], BF16, tag="wo")
        # per-qt sums live in row 0 (rows 1-127 permanently zero) so the
        # norm's broadcast matmul gets a full 128-contraction stationary whose
        # LDWEIGHTS hides, unlike a 1-partition ones row; ping-pong on qt%2
        sums_pad = acts.tile([128, 2, 2, 512], BF16, tag="sums_pad")
        nc.gpsimd.memset(sums_pad[:], 0.0)

        # DMA issue order matters: each dma_start costs ~650ns serially on the
        # Sync engine, so the prologue-critical transfers (W_K, then xT for
        # g=0) must be triggered first.
        xT_src = xT_dram.rearrange("(kc p) s -> p kc s", p=128)
        nc.sync.dma_start(wk_sb[:], wkT_dram.rearrange("(kc p) h -> p kc h", p=128))
        for kc in range(n_kc):
            nc.sync.dma_start(xT[:, kc, ts(0, QT)], xT_src[:, kc, ts(0, QT)])
        nc.sync.dma_start(wq_sb[:], wqT_dram.rearrange("(kc p) h -> p kc h", p=128))
        nc.sync.dma_start(wv_sb[:], wvT_dram.rearrange("(kc p) h -> p kc h", p=128))
        nc.sync.dma_start(wo_sb[:], woT_dram[:])
        for g in range(1, nqt):
            for kc in range(n_kc):
                nc.sync.dma_start(xT[:, kc, ts(g, QT)], xT_src[:, kc, ts(g, QT)])
        ones_cols = vtiles[:, :, :].rearrange("p c (g hh) -> p c g hh", g=2)[:, :, :, dh : dh + 1]
        nc.gpsimd.memset(ones_cols, 1.0)

        def build_k_mm(g, pj, kc_pair):
            """Half of the K^T projection matmuls for p-slice g."""
            for kc in (2 * kc_pair, 2 * kc_pair + 1):
                nc.tensor.matmul(
                    pj[:, :], wk_sb[:, kc, :], xT[:, kc, ts(g, QT)],
                    start=(kc == 0), stop=(kc == n_kc - 1),
                )

        def build_k_fin(g, pj):
            nc.vector.tensor_copy(kTA[0:dh, ts(g, QT)], pj[0:dh, :])
            nc.vector.tensor_copy(kTB[ds(dh, dh), ts(g, QT)], pj[ds(dh, dh), :])

        def build_k(g):
            pj = pmp.tile([128, 512], F32, tag="pmisc", name=f"pjk{g}")
            build_k_mm(g, pj, 0)
            build_k_mm(g, pj, 1)
            build_k_fin(g, pj)

        def build_q(g):
            pj = pmp.tile([128, 512], F32, tag="pmisc", name=f"pjq{g}")
            for kc in range(n_kc):
                nc.tensor.matmul(
                    pj[:, :], wq_sb[:, kc, :], xT[:, kc, ts(g, QT)],
                    start=(kc == 0), stop=(kc == n_kc - 1),
                )
            nc.vector.tensor_copy(qT[:, ts(g, QT)], pj[:, :])

        def build_v(g):
            """Build V tiles for p-slice [512g, 512g+512)."""
            pj = pmp.tile([128, 512], F32, tag="pmisc", name=f"pjv{g}")
            for kc in range(n_kc):
                nc.tensor.matmul(
                    pj[:, :], wv_sb[:, kc, :], xT[:, kc, ts(g, QT)],
                    start=(kc == 0), stop=(kc == n_kc - 1),
                )
            vts = vstage.tile([128, 512], BF16, tag="vts", name=f"vts{g}")
            nc.vector.tensor_copy(vts[:], pj[:, :])
            vtp = pmp.tile([128, 512], BF16, tag="pmisc", name=f"vtp{g}")
            for i in range(4):
                nc.tensor.transpose(vtp[:, ts(i, 128)], vts[:, ts(i, 128)], ident_bf[:])
            # one strided copy: [p, chunk, head, 64] -> vtiles head segments
            src = vtp[:].rearrange("p (c g2 hh) -> p c g2 hh", c=4, g2=2)
            dst = vtiles[:, ds(4 * g, 4), :].rearrange(
                "p c (g2 x) -> p c g2 x", g2=2
            )[:, :, :, 0:dh]
            nc.vector.tensor_copy(dst, src)

        fillers = deque()  # one deferred PE task consumed per block

        def epi_chunk(qt, zun, qc):
            # one 128-row chunk of the deferred output projection
            ops = pmp.tile([128, 512], F32, tag="pmisc", name=f"ops{qt}_{qc}")
            nc.tensor.matmul(
                ops[:, :], zun[:, ts(qc, 128)], wo_sb[:, :],
                start=True, stop=True,
            )
            osb = ost.tile([128, d], BF16, tag="ob", name=f"ob{qt}_{qc}")
            if qt == nqt - 1 and qc % 2 == 0:
                # tail: split the final casts across Scalar and Vector
                nc.scalar.copy(osb[:], ops[:, :])
            else:
                nc.vector.tensor_copy(osb[:], ops[:, :])
            nc.sync.dma_start(out_dram[ds(QT * qt + 128 * qc, 128), :], osb[:])

        def make_norm(qt, zsb):
            def norm():
                # broadcast sums into rows via full-contraction PE matmuls
                # (all-ones stationary x zero-padded sums), then reciprocal +
                # one full-width scale of z^T
                rr = pmp.tile([128, 512], F32, tag="pmisc", name=f"rr{qt}")
                nc.tensor.matmul(
                    rr[0:dh, :], ones128[:, :], sums_pad[:, qt % 2, 0, :],
                    start=True, stop=True,
                )
                nc.tensor.matmul(
                    rr[ds(dh, dh), :], ones128[:, :], sums_pad[:, qt % 2, 1, :],
                    start=True, stop=True, tile_position=(0, dh),
                )
                rcp = nrm.tile([128, 512], F32, tag="rcp", name=f"rcp{qt}")
                nc.vector.reciprocal_approx_fast(rcp[:], rr[:, :])
                zun = nrm.tile([128, 512], BF16, tag="zun", name=f"zun{qt}")
                if qt == nqt - 1:
                    # tail: scale z^T in 128-col chunks so each output chunk's
                    # projection starts as soon as its slice is ready
                    for qc in range(4):
                        nc.vector.tensor_mul(
                            zun[:, ts(qc, 128)], zsb[:, ts(qc, 128)], rcp[:, ts(qc, 128)]
                        )
                        epi_chunk(qt, zun, qc)
                else:
                    nc.vector.tensor_mul(zun[:], zsb[:], rcp[:])
                    fillers.extend(
                        (lambda q=qt, z=zun, c=qc: epi_chunk(q, z, c))
                        for qc in range(4)
                    )
            return norm

        # ---- prologue: K0/Q0/V0 (K1 is built in-stream during qt=0).
        # K0/Q0 interleave per k-chunk so each arriving xT DMA chunk feeds
        # two matmuls instead of one ----
        pjk0 = pmp.tile([128, 512], F32, tag="pmisc", name="pjk0")
        pjq0 = pmp.tile([128, 512], F32, tag="pmisc", name="pjq0")
        for kc in range(n_kc):
            nc.tensor.matmul(
                pjk0[:, :], wk_sb[:, kc, :], xT[:, kc, ts(0, QT)],
                start=(kc == 0), stop=(kc == n_kc - 1),
            )
            nc.tensor.matmul(
                pjq0[:, :], wq_sb[:, kc, :], xT[:, kc, ts(0, QT)],
                start=(kc == 0), stop=(kc == n_kc - 1),
            )
        build_k_fin(0, pjk0)
        nc.vector.tensor_copy(qT[:, ts(0, QT)], pjq0[:, :])
        build_v(0)

        # ---- flat block pipeline: one stream over all (qt, pc) so neither
        # engine drains at q-tile boundaries ----
        n_pc_of = lambda qt: (QT // PC) * (qt + 1)
        zps_of = {}
        norm_of = {}

        def emit_pv(pv_qt, pv_pc, pv_pt, pv_w):
            if pv_pc == 0:
                zps_of[pv_qt] = pzp.tile(
                    [dh + 1, 1024], F32, tag="zps", name=f"zps{pv_qt}"
                )
            zps = zps_of[pv_qt]
            off = 512 - pv_w
            for h in range(2):
                nc.tensor.matmul(
                    zps[:, ds(512 * h + off, pv_w)],
                    vtiles[:, pv_pc, ds((dh + 1) * h, dh + 1)],
                    pv_pt[:, ds(512 * h, pv_w)],
                    start=(pv_pc == 0), stop=(pv_pc == n_pc_of(pv_qt) - 1),
                )
            if pv_pc == n_pc_of(pv_qt) - 1:
                # drain zps fast: the sums row first (the norm's ones-matmul
                # waits on it), then z^T halves into separate partition ranges
                # (so the later scale is one same-base multiply)
                for h in range(2):
                    nc.vector.tensor_copy(
                        sums_pad[0:1, pv_qt % 2, h, :], zps[dh : dh + 1, ts(h, 512)]
                    )
                zsb = nrm.tile([128, 512], BF16, tag="zsb", name=f"zsb{pv_qt}")
                for h in range(2):
                    nc.vector.tensor_copy(
                        zsb[ds(dh * h, dh), :], zps[0:dh, ts(h, 512)]
                    )
                norm_of[pv_qt] = make_norm(pv_qt, zsb)

        pv_pending = deque()
        PV_LAG = 3  # PV of block k emitted during block k+3
        for qt in range(nqt):
            n_pc = n_pc_of(qt)
            if qt + 1 < nqt:
                fillers.append(lambda g=qt + 1: build_q(g))
                fillers.append(lambda g=qt + 1: build_v(g))
            # K(qt+1) as one boundary burst: the PE has slack here (ACT is
            # still finishing qt's exps) and its DVE casts clear the queue
            # before the zps drain needs it
            if qt + 1 < nqt:
                build_k(qt + 1)
            for pc in range(n_pc):
                # late enough that the fillers' pmp ring has drained (the
                # norm's ones-matmuls otherwise wait on a pool slot)
                norm_pc = 3 if qt == nqt - 1 else (7 if qt == 1 else 10)
                if pc == norm_pc and (qt - 1) in norm_of:
                    norm_of.pop(qt - 1)()
                # hold back fillers at the end of qt = nqt-2 so the filler-less
                # early blocks of the last q-tile keep the PE fed
                consume = pc >= 4 or (qt == 0 and pc >= 2) or (qt == nqt - 1 and pc >= 1)
                if qt == nqt - 2 and pc >= 8:
                    consume = False
                if consume and fillers:
                    fillers.popleft()()
                j = pc - (QT // PC) * qt
                w = 512 if j < 0 else 512 - 128 * j
                qg = QT * qt + (512 - w)
                # heads at fixed 512-col offsets (PSUM-bank aligned)
                sps = psp.tile([128, 1024], F32, tag="sps", name=f"sps{qt}_{pc}")
                for h, kt in enumerate((kTA, kTB)):
                    nc.tensor.matmul(
                        sps[:, ds(512 * h, w)],
                        kt[:, ts(pc, 128)],
                        qT[:, ds(qg, w)],
                        start=True, stop=True,
                    )
                pt_sb = ptp.tile([128, 1024], BF16, tag="pt", name=f"pt{qt}_{pc}")
                sps_v = sps[:].rearrange("p (g c) -> p g c", g=2)[:, :, 0:w]
                pt_v = pt_sb[:].rearrange("p (g c) -> p g c", g=2)[:, :, 0:w]
                nc.scalar.activation(
                    pt_v, sps_v, mybir.ActivationFunctionType.Exp,
                    bias=0.0, scale=1.0 / np.sqrt(dh).item(),
                )
                if j >= 0:  # diagonal 128-block: zero non-causal probs
                    for h in range(2):
                        nc.vector.tensor_mul(
                            pt_sb[:, ds(512 * h, 128)], pt_sb[:, ds(512 * h, 128)], dmask[:, :]
                        )
                if len(pv_pending) >= PV_LAG:
                    emit_pv(*pv_pending.popleft())
                pv_pending.append((qt, pc, pt_sb, w))
        while pv_pending:
            emit_pv(*pv_pending.popleft())
        norm_of.pop(nqt - 1)()
        while fillers:
            fillers.popleft()()

    nc.finalize()
    return nc


_NC_CACHE = {}


def _get_nc(s=S):
    if s not in _NC_CACHE:
        _NC_CACHE[s] = build_attention_core(s=s)
    return _NC_CACHE[s]


def make_in_maps(x, W_K, W_Q, W_V, W_O):
    bf = ml_dtypes.bfloat16
    in_maps = []
    for c in range(N_CORES):
        b, hp = c // 4, c % 4
        hA, hB = 2 * hp, 2 * hp + 1
        wkT = np.concatenate([W_K[hA].T, W_K[hB].T], axis=1).astype(bf)  # [d, 128]
        wqT = np.concatenate([W_Q[hA].T, W_Q[hB].T], axis=1).astype(bf)
        wvT = np.concatenate([W_V[hA].T, W_V[hB].T], axis=1).astype(bf)
        woT = np.ascontiguousarray(W_O[:, DH * hA : DH * (hB + 1)].T).astype(bf)  # [128, d]
        xT = np.ascontiguousarray(np.asarray(x[b], dtype=np.float32).T).astype(bf)  # [d, s]
        in_maps.append(
            {
                "xT": xT,
                "wkT": np.ascontiguousarray(wkT),
                "wqT": np.ascontiguousarray(wqT),
                "wvT": np.ascontiguousarray(wvT),
                "woT": woT,
            }
        )
    return in_maps


def kernel(x, W_K, W_Q, W_V, W_O):
    from concourse.bass_utils import run_bass_kernel_spmd

    nc = _get_nc(S)
    in_maps = make_in_maps(x, W_K, W_Q, W_V, W_O)
    res = run_bass_kernel_spmd(nc, in_maps, core_ids=list(range(N_CORES)))
    out = np.zeros((B, S, D), dtype=np.float32)
    for c in range(N_CORES):
        out[c // 4] += np.asarray(res.results[c]["out"], dtype=np.float32)
    return out


# revision 50
# speedup vs baseline: 1.0130x; 1.0130x over previous
"""Trainium2 Bass kernel for 8-head causal self-attention (b=2, s=4096, d=512, 8 heads x 64).

Sharding: 8 cores = 2 (batch) x 4 (head-pair). Core c handles batch c//4 and heads
(2*(c%4), 2*(c%4)+1). Each core computes a partial output projection over its two
heads' columns of W_O; the host sums the 4 partials per batch (tensor-parallel
all-reduce done on host at gather time).

Final structure (v15): one flat software-pipelined stream over all 144
(q-tile, p-block) blocks; K^T stored zero-padded per head (kTA/kTB) so score
matmuls run at full 128-contraction rate; PV runs 3 blocks behind its block's
exp (PV_LAG) so it never touches the pt tile ACT is writing; the PV-accumulator
PSUM tile is drained right after the last PV matmul (sums row first, then two
partition-shifted DVE casts) so the next q-tile's PV never waits on the single
zps buffer; the rest of the softmax normalization (ones-row PE broadcast of
sums, fast reciprocal, one full-width scale) is deferred into block 7 of the
next q-tile; K(qt+1) is projected as one boundary burst; Q/V builds for qt+1
and the previous tile's output projection are spread one-per-block through a
persistent filler queue; DMA issue order puts W_K and xT g=0 first (each
dma_start costs ~650ns serially on the Sync engine).

Per-core algorithm ("everything transposed", softmax over the partition axis):
  - K^T/Q^T projections from xT (2 heads packed), V re-transposed to [p, 65]
    tiles with a fused ones-column (softmax sums ride the PV matmul)
  - S^T[p,q] blocks via row-tiled matmuls, exp on ScalarE (scale=1/8),
    causal 128x128 diagonal masked post-exp with one triangular bf16 mask
  - PV matmul accumulates z^T (+ sums row via the ones-column) in PSUM
"""

import numpy as np
import ml_dtypes
from collections import deque
from contextlib import ExitStack

import concourse.bass as bass
import concourse.mybir as mybir
import concourse.tile as tile
from concourse import bacc
from concourse.bass import ts, ds
from concourse.masks import make_identity

BF16 = mybir.dt.bfloat16
F32 = mybir.dt.float32

B, S, D, NH, DH = 2, 4096, 512, 8, 64
N_CORES = 8
QT = 512          # q tile (free dim of S^T blocks)
PC = 128          # p chunk (partition dim of S^T blocks)


def build_attention_core(s=S, d=D, dh=DH):
    nqt = s // QT
    n_kc = d // 128
    nc = bacc.Bacc()
    xT_dram = nc.dram_tensor("xT", [d, s], BF16, kind="ExternalInput")
    wkT_dram = nc.dram_tensor("wkT", [d, 2 * dh], BF16, kind="ExternalInput")
    wqT_dram = nc.dram_tensor("wqT", [d, 2 * dh], BF16, kind="ExternalInput")
    wvT_dram = nc.dram_tensor("wvT", [d, 2 * dh], BF16, kind="ExternalInput")
    woT_dram = nc.dram_tensor("woT", [2 * dh, d], BF16, kind="ExternalInput")
    out_dram = nc.dram_tensor("out", [s, d], BF16, kind="ExternalOutput")

    with ExitStack() as ctx:
        tc = ctx.enter_context(tile.TileContext(nc))
        consts = ctx.enter_context(tc.tile_pool(name="consts", bufs=1))
        acts = ctx.enter_context(tc.tile_pool(name="acts", bufs=1))
        vstage = ctx.enter_context(tc.tile_pool(name="vstage", bufs=2))
        ptp = ctx.enter_context(tc.tile_pool(name="ptp", bufs=5))
        nrm = ctx.enter_context(tc.tile_pool(name="nrm", bufs=3))
        ost = ctx.enter_context(tc.tile_pool(name="ost", bufs=4))
        psp = ctx.enter_context(tc.tile_pool(name="psp", bufs=2, space="PSUM"))
        pzp = ctx.enter_context(tc.tile_pool(name="pzp", bufs=1, space="PSUM"))
        pmp = ctx.enter_context(tc.tile_pool(name="pmp", bufs=2, space="PSUM"))

        # ---- constants ----
        ident_bf = consts.tile([128, 128], BF16, tag="idb")
        make_identity(nc, ident_bf[:])
        ones128 = consts.tile([128, dh], BF16, tag="ones")
        nc.gpsimd.memset(ones128[:], 1.0)
        # diagonal 128x128 causal mask: keep[p, q] = 1 if q >= p
        dmask = consts.tile([128, 128], BF16, tag="dgm")
        nc.gpsimd.memset(dmask[:], 1.0)
        nc.gpsimd.affine_select(
            out=dmask[:], in_=dmask[:],
            compare_op=mybir.AluOpType.is_ge,
            fill=0.0, base=0,
            pattern=[[1, 128]], channel_multiplier=-1,
        )

        # ---- persistent activations / weights ----
        xT = acts.tile([128, n_kc, s], BF16, tag="xT")
        # K^T stored zero-padded per head so score matmuls run at full
        # 128-contraction rate: kTA rows 0-63 = K_A (rows 64-127 zero),
        # kTB rows 64-127 = K_B (rows 0-63 zero). The zero rows multiply
        # the other head's rows of qT and contribute nothing.
        kTA = acts.tile([128, s], BF16, tag="kTA")
        kTB = acts.tile([128, s], BF16, tag="kTB")
        nc.vector.memset(kTA[ds(dh, dh), :], 0.0)
        nc.gpsimd.memset(kTB[0:dh, :], 0.0)
        qT = acts.tile([128, s], BF16, tag="qT")
        vtiles = acts.tile([128, s // PC, 2 * (dh + 1)], BF16, tag="vt")
        wk_sb = acts.tile([128, n_kc, 2 * dh], BF16, tag="wk")
        wq_sb = acts.tile([128, n_kc, 2 * dh], BF16, tag="wq")
        wv_sb = acts.tile([128, n_kc, 2 * dh], BF16, tag="wv")
        wo_sb = acts.tile([128, d], BF16, tag="wo")
        # per-qt sums live in row 0 (rows 1-127 permanently zero) so the
        # norm's broadcast matmul gets a full 128-contraction stationary whose
        # LDWEIGHTS hides, unlike a 1-partition ones row; ping-pong on qt%2
        sums_pad = acts.tile([128, 2, 2, 512], BF16, tag="sums_pad")
        nc.gpsimd.memset(sums_pad[:], 0.0)

        # DMA issue order matters: each dma_start costs ~650ns serially on the
        # Sync engine, so the prologue-critical transfers (W_K, then xT for
        # g=0) must be triggered first.
        xT_src = xT_dram.rearrange("(kc p) s -> p kc s", p=128)
        nc.sync.dma_start(wk_sb[:], wkT_dram.rearrange("(kc p) h -> p kc h", p=128))
        for kc in range(n_kc):
            nc.sync.dma_start(xT[:, kc, ts(0, QT)], xT_src[:, kc, ts(0, QT)])
        nc.sync.dma_start(wq_sb[:], wqT_dram.rearrange("(kc p) h -> p kc h", p=128))
        nc.sync.dma_start(wv_sb[:], wvT_dram.rearrange("(kc p) h -> p kc h", p=128))
        nc.sync.dma_start(wo_sb[:], woT_dram[:])
        for g in range(1, nqt):
            for kc in range(n_kc):
                nc.sync.dma_start(xT[:, kc, ts(g, QT)], xT_src[:, kc, ts(g, QT)])
        ones_cols = vtiles[:, :, :].rearrange("p c (g hh) -> p c g hh", g=2)[:, :, :, dh : dh + 1]
        nc.gpsimd.memset(ones_cols, 1.0)

        def build_k_mm(g, pj, kc_pair):
            """Half of the K^T projection matmuls for p-slice g."""
            for kc in (2 * kc_pair, 2 * kc_pair + 1):
                nc.tensor.matmul(
                    pj[:, :], wk_sb[:, kc, :], xT[:, kc, ts(g, QT)],
                    start=(kc == 0), stop=(kc == n_kc - 1),
                )

        def build_k_fin(g, pj):
            nc.vector.tensor_copy(kTA[0:dh, ts(g, QT)], pj[0:dh, :])
            nc.vector.tensor_copy(kTB[ds(dh, dh), ts(g, QT)], pj[ds(dh, dh), :])

        def build_k(g):
            pj = pmp.tile([128, 512], F32, tag="pmisc", name=f"pjk{g}")
            build_k_mm(g, pj, 0)
            build_k_mm(g, pj, 1)
            build_k_fin(g, pj)

        def build_q(g):
            pj = pmp.tile([128, 512], F32, tag="pmisc", name=f"pjq{g}")
            for kc in range(n_kc):
                nc.tensor.matmul(
                    pj[:, :], wq_sb[:, kc, :], xT[:, kc, ts(g, QT)],
                    start=(kc == 0), stop=(kc == n_kc - 1),
                )
            nc.vector.tensor_copy(qT[:, ts(g, QT)], pj[:, :])

        def build_v(g):
            """Build V tiles for p-slice [512g, 512g+512)."""
            pj = pmp.tile([128, 512], F32, tag="pmisc", name=f"pjv{g}")
            for kc in range(n_kc):
                nc.tensor.matmul(
                    pj[:, :], wv_sb[:, kc, :], xT[:, kc, ts(g, QT)],
                    start=(kc == 0), stop=(kc == n_kc - 1),
                )
            vts = vstage.tile([128, 512], BF16, tag="vts", name=f"vts{g}")
            nc.vector.tensor_copy(vts[:], pj[:, :])
            vtp = pmp.tile([128, 512], BF16, tag="pmisc", name=f"vtp{g}")
            for i in range(4):
                nc.tensor.transpose(vtp[:, ts(i, 128)], vts[:, ts(i, 128)], ident_bf[:])
            # one strided copy: [p, chunk, head, 64] -> vtiles head segments
            src = vtp[:].rearrange("p (c g2 hh) -> p c g2 hh", c=4, g2=2)
            dst = vtiles[:, ds(4 * g, 4), :].rearrange(
                "p c (g2 x) -> p c g2 x", g2=2
            )[:, :, :, 0:dh]
            nc.vector.tensor_copy(dst, src)

        fillers = deque()  # one deferred PE task consumed per block

        def epi_chunk(qt, zun, qc):
            # one 128-row chunk of the deferred output projection
            ops = pmp.tile([128, 512], F32, tag="pmisc", name=f"ops{qt}_{qc}")
            nc.tensor.matmul(
                ops[:, :], zun[:, ts(qc, 128)], wo_sb[:, :],
                start=True, stop=True,
            )
            osb = ost.tile([128, d], BF16, tag="ob", name=f"ob{qt}_{qc}")
            if qt == nqt - 1 and qc % 2 == 0:
                # tail: split the final casts across Scalar and Vector
                nc.scalar.copy(osb[:], ops[:, :])
            else:
                nc.vector.tensor_copy(osb[:], ops[:, :])
            nc.sync.dma_start(out_dram[ds(QT * qt + 128 * qc, 128), :], osb[:])

        def make_norm(qt, zsb):
            def norm():
                # broadcast sums into rows via full-contraction PE matmuls
                # (all-ones stationary x zero-padded sums), then reciprocal +
                # one full-width scale of z^T
                rr = pmp.tile([128, 512], F32, tag="pmisc", name=f"rr{qt}")
                nc.tensor.matmul(
                    rr[0:dh, :], ones128[:, :], sums_pad[:, qt % 2, 0, :],
                    start=True, stop=True,
                )
                nc.tensor.matmul(
                    rr[ds(dh, dh), :], ones128[:, :], sums_pad[:, qt % 2, 1, :],
                    start=True, stop=True, tile_position=(0, dh),
                )
                rcp = nrm.tile([128, 512], F32, tag="rcp", name=f"rcp{qt}")
                nc.vector.reciprocal_approx_fast(rcp[:], rr[:, :])
                zun = nrm.tile([128, 512], BF16, tag="zun", name=f"zun{qt}")
                if qt == nqt - 1:
                    # tail: scale z^T in 128-col chunks so each output chunk's
                    # projection starts as soon as its slice is ready
                    for qc in range(4):
                        nc.vector.tensor_mul(
                            zun[:, ts(qc, 128)], zsb[:, ts(qc, 128)], rcp[:, ts(qc, 128)]
                        )
                        epi_chunk(qt, zun, qc)
                else:
                    nc.vector.tensor_mul(zun[:], zsb[:], rcp[:])
                    fillers.extend(
                        (lambda q=qt, z=zun, c=qc: epi_chunk(q, z, c))
                        for qc in range(4)
                    )
            return norm

        # ---- prologue: K0/Q0/V0 (K1 is built in-stream during qt=0) ----
        build_k(0)
        build_q(0)
        build_v(0)

        # ---- flat block pipeline: one stream over all (qt, pc) so neither
        # engine drains at q-tile boundaries ----
        n_pc_of = lambda qt: (QT // PC) * (qt + 1)
        zps_of = {}
        norm_of = {}

        def emit_pv(pv_qt, pv_pc, pv_pt, pv_w):
            if pv_pc == 0:
                zps_of[pv_qt] = pzp.tile(
                    [dh + 1, 1024], F32, tag="zps", name=f"zps{pv_qt}"
                )
            zps = zps_of[pv_qt]
            off = 512 - pv_w
            for h in range(2):
                nc.tensor.matmul(
                    zps[:, ds(512 * h + off, pv_w)],
                    vtiles[:, pv_pc, ds((dh + 1) * h, dh + 1)],
                    pv_pt[:, ds(512 * h, pv_w)],
                    start=(pv_pc == 0), stop=(pv_pc == n_pc_of(pv_qt) - 1),
                )
            if pv_pc == n_pc_of(pv_qt) - 1:
                # drain zps fast: the sums row first (the norm's ones-matmul
                # waits on it), then z^T halves into separate partition ranges
                # (so the later scale is one same-base multiply)
                for h in range(2):
                    nc.vector.tensor_copy(
                        sums_pad[0:1, pv_qt % 2, h, :], zps[dh : dh + 1, ts(h, 512)]
                    )
                zsb = nrm.tile([128, 512], BF16, tag="zsb", name=f"zsb{pv_qt}")
                for h in range(2):
                    nc.vector.tensor_copy(
                        zsb[ds(dh * h, dh), :], zps[0:dh, ts(h, 512)]
                    )
                norm_of[pv_qt] = make_norm(pv_qt, zsb)

        pv_pending = deque()
        PV_LAG = 3  # PV of block k emitted during block k+3
        for qt in range(nqt):
            n_pc = n_pc_of(qt)
            if qt + 1 < nqt:
                fillers.append(lambda g=qt + 1: build_q(g))
                fillers.append(lambda g=qt + 1: build_v(g))
            # K(qt+1) as one boundary burst: the PE has slack here (ACT is
            # still finishing qt's exps) and its DVE casts clear the queue
            # before the zps drain needs it
            if qt + 1 < nqt:
                build_k(qt + 1)
            for pc in range(n_pc):
                # late enough that the fillers' pmp ring has drained (the
                # norm's ones-matmuls otherwise wait on a pool slot)
                norm_pc = 3 if qt == nqt - 1 else (7 if qt == 1 else 10)
                if pc == norm_pc and (qt - 1) in norm_of:
                    norm_of.pop(qt - 1)()
                # hold back fillers at the end of qt = nqt-2 so the filler-less
                # early blocks of the last q-tile keep the PE fed
                consume = pc >= 4 or (qt == 0 and pc >= 2) or (qt == nqt - 1 and pc >= 1)
                if qt == nqt - 2 and pc >= 8:
                    consume = False
                if consume and fillers:
                    fillers.popleft()()
                j = pc - (QT // PC) * qt
                w = 512 if j < 0 else 512 - 128 * j
                qg = QT * qt + (512 - w)
                # heads at fixed 512-col offsets (PSUM-bank aligned)
                sps = psp.tile([128, 1024], F32, tag="sps", name=f"sps{qt}_{pc}")
                for h, kt in enumerate((kTA, kTB)):
                    nc.tensor.matmul(
                        sps[:, ds(512 * h, w)],
                        kt[:, ts(pc, 128)],
                        qT[:, ds(qg, w)],
                        start=True, stop=True,
                    )
                pt_sb = ptp.tile([128, 1024], BF16, tag="pt", name=f"pt{qt}_{pc}")
                sps_v = sps[:].rearrange("p (g c) -> p g c", g=2)[:, :, 0:w]
                pt_v = pt_sb[:].rearrange("p (g c) -> p g c", g=2)[:, :, 0:w]
                nc.scalar.activation(
                    pt_v, sps_v, mybir.ActivationFunctionType.Exp,
                    bias=0.0, scale=1.0 / np.sqrt(dh).item(),
                )
                if j >= 0:  # diagonal 128-block: zero non-causal probs
                    for h in range(2):
                        nc.vector.tensor_mul(
                            pt_sb[:, ds(512 * h, 128)], pt_sb[:, ds(512 * h, 128)], dmask[:, :]
                        )
                if len(pv_pending) >= PV_LAG:
                    emit_pv(*pv_pending.popleft())
                pv_pending.append((qt, pc, pt_sb, w))
        while pv_pending:
            emit_pv(*pv_pending.popleft())
        norm_of.pop(nqt - 1)()
        while fillers:
            fillers.popleft()()

    nc.finalize()
    return nc


_NC_CACHE = {}


def _get_nc(s=S):
    if s not in _NC_CACHE:
        _NC_CACHE[s] = build_attention_core(s=s)
    return _NC_CACHE[s]


def make_in_maps(x, W_K, W_Q, W_V, W_O):
    bf = ml_dtypes.bfloat16
    in_maps = []
    for c in range(N_CORES):
        b, hp = c // 4, c % 4
        hA, hB = 2 * hp, 2 * hp + 1
        wkT = np.concatenate([W_K[hA].T, W_K[hB].T], axis=1).astype(bf)  # [d, 128]
        wqT = np.concatenate([W_Q[hA].T, W_Q[hB].T], axis=1).astype(bf)
        wvT = np.concatenate([W_V[hA].T, W_V[hB].T], axis=1).astype(bf)
        woT = np.ascontiguousarray(W_O[:, DH * hA : DH * (hB + 1)].T).astype(bf)  # [128, d]
        xT = np.ascontiguousarray(np.asarray(x[b], dtype=np.float32).T).astype(bf)  # [d, s]
        in_maps.append(
            {
                "xT": xT,
                "wkT": np.ascontiguousarray(wkT),
                "wqT": np.ascontiguousarray(wqT),
                "wvT": np.ascontiguousarray(wvT),
                "woT": woT,
            }
        )
    return in_maps


def kernel(x, W_K, W_Q, W_V, W_O):
    from concourse.bass_utils import run_bass_kernel_spmd

    nc = _get_nc(S)
    in_maps = make_in_maps(x, W_K, W_Q, W_V, W_O)
    res = run_bass_kernel_spmd(nc, in_maps, core_ids=list(range(N_CORES)))
    out = np.zeros((B, S, D), dtype=np.float32)
    for c in range(N_CORES):
        out[c // 4] += np.asarray(res.results[c]["out"], dtype=np.float32)
    return out
